# revision 48
# baseline (speedup 1.0000x reference)
"""Self-contained Trainium2 kernel for nn_Attention_22814866276679.

Multi-head attention (ViT-style, 197 tokens, 12 heads, dim 768) with a
relative-position bias table, batch 64. Data-parallel over batch across the
8 NeuronCores (8 images per core, no collectives).

Math notes (host prep moves all layout work off the device):
  - qkv = x @ w_qkv.T + concat(q_bias, 0, v_bias); q *= 1/8. The 1/8 scale
    and the biases are folded into pre-transposed weight matrices with an
    extra contraction row (x gets a ones row).
  - scores are computed TRANSPOSED ([keys, queries]) so the softmax reduce
    (over keys) lands on the matmul contraction axis; no PE transposes.
  - |scores + bias| <= ~3 for these inputs, so exp() is computed without the
    max-subtraction (mathematically identical softmax).
  - V carries an appended ones column: the attention@V matmul then emits the
    softmax denominators as a 65th output row for free.
"""

import os
import sys

for _p in ("/opt/trn_rl_repo", "/root/.axon_site/_ro/trn_rl_repo"):
    if os.path.isdir(_p) and _p not in sys.path:
        sys.path.insert(0, _p)

import ml_dtypes
import numpy as np

import concourse.bass as bass
import concourse.mybir as mybir
import concourse.tile as tile
from concourse import bacc, library_config
from concourse.masks import make_identity

BF16 = mybir.dt.bfloat16
F32 = mybir.dt.float32

B, N, DIM, H, HD = 64, 197, 768, 12, 64
NCORES = 8
BL = B // NCORES          # 8 images per core
TOK = BL * N              # 1576 tokens per core
C = 7                     # contraction chunks of 128 (768 dims + bias row, padded)
CP = C * 128              # 896
NQ = 394                  # qk-projection free chunk (4 * 394 = 1576)
PRJ = 384                 # v / output-projection free chunk (2 * 384 = 768)
N2 = 2 * N                # paired scores free size (keys 0:128 | keys 128:197)
FT = 2 * DIM // 128       # 12 q/k feature tiles (0-5: q, 6-11: k)

MUL = mybir.AluOpType.mult


def build_module(debug_taps: bool = False) -> bass.Bass:
    nc = bacc.Bacc()
    xt_d = nc.declare_dram_parameter("xt", [CP, TOK], BF16, isOutput=False)
    wqk_d = nc.declare_dram_parameter("wqk", [CP, 2 * DIM], BF16, isOutput=False)
    wv_d = nc.declare_dram_parameter("wv", [CP, DIM], BF16, isOutput=False)
    wp_d = nc.declare_dram_parameter("wp", [CP, DIM], BF16, isOutput=False)
    bp_d = nc.declare_dram_parameter("bpair", [128, H, N2], BF16, isOutput=False)
    eb_d = nc.declare_dram_parameter("ebpair", [128, H, N2], BF16, isOutput=False)
    out_d = nc.declare_dram_parameter("out", [TOK, DIM], F32, isOutput=True)
    if debug_taps:
        dbg_qkT = nc.declare_dram_parameter("dbg_qkT", [128, FT, TOK], BF16, isOutput=True)
        dbg_vst = nc.declare_dram_parameter("dbg_vst", [128, 2 * BL, H, HD + 1], BF16, isOutput=True)
        dbg_aoT = nc.declare_dram_parameter("dbg_aoT", [128, C, TOK], BF16, isOutput=True)
        dbg_e = nc.declare_dram_parameter("dbg_e", [128, N2], BF16, isOutput=True)
        dbg_r = nc.declare_dram_parameter("dbg_r", [128, N], F32, isOutput=True)

    with tile.TileContext(nc) as tc:
        with (
            tc.tile_pool(name="persist", bufs=1) as persist,
            tc.tile_pool(name="sb_e", bufs=6) as sb_e,
            tc.tile_pool(name="sb_r", bufs=8) as sb_r,
            tc.tile_pool(name="sb_rb", bufs=8) as sb_rb,
            tc.tile_pool(name="sb_out", bufs=4) as sb_out,
        ):
            xt = persist.tile([128, C, TOK], BF16)
            wqk = persist.tile([128, C, 2 * DIM], BF16)
            wv = persist.tile([128, C, DIM], BF16)
            wp = persist.tile([128, C, DIM], BF16)
            bp = persist.tile([128, H, N2], BF16)
            eb = persist.tile([128, H, N2], BF16)
            # f 0-5: qT, 6-11: kT; +64 zero tail columns let the second
            # scores matmul always run M=128 (keys q0+128 .. q0+256)
            qkT = persist.tile([128, FT, TOK + 64], BF16)
            vst = persist.tile([128, 2 * BL, H, HD + 1], BF16)
            aoT = persist.tile([128, C, TOK], BF16)

            # partition_broadcast + gpsimd tensor_tensor live in 'proxy'
            nc.gpsimd.load_library(library_config.proxy)
            if debug_taps:
                nc.gpsimd.memset(vst[:], 0.0)
            # ones row (contraction row 768) for the output projection bias
            nc.gpsimd.memset(aoT[:, 6, :], 0.0)
            nc.gpsimd.memset(aoT[0:1, 6, :], 1.0)
            nc.gpsimd.memset(qkT[:, :, TOK:TOK + 64], 0.0)

            # per-chunk DMAs, interleaved so the first qk matmuls (which need
            # xt[c] + wqk[c]) can start as soon as their chunk lands
            for c in range(C):
                nc.sync.dma_start(xt[:, c, :], xt_d[c * 128:(c + 1) * 128, :])
                nc.sync.dma_start(wqk[:, c, :], wqk_d[c * 128:(c + 1) * 128, :])
            for c in range(C):
                nc.sync.dma_start(wv[:, c, :], wv_d[c * 128:(c + 1) * 128, :])
            nc.sync.dma_start(bp[:], bp_d[:])
            nc.sync.dma_start(eb[:], eb_d[:])
            for c in range(C):
                nc.sync.dma_start(wp[:, c, :], wp_d[c * 128:(c + 1) * 128, :])

            # ---- q/k projections, feature-major: qkT[f] = w[f-block] @ x.T
            with tc.tile_pool(name="ps_qk", bufs=4, space="PSUM") as ps_qk:
                for f in range(FT):
                    # k features (f >= 6) have no bias: row 768 of wqk is zero
                    # there, so the c == 6 (bias/padding) chunk is skippable
                    cs = C if f < FT // 2 else C - 1
                    for n in range(TOK // NQ):
                        ps = ps_qk.tile([128, NQ], F32)
                        for c in range(cs):
                            nc.tensor.matmul(
                                ps[:, :],
                                lhsT=wqk[:, c, f * 128:(f + 1) * 128],
                                rhs=xt[:, c, n * NQ:(n + 1) * NQ],
                                start=(c == 0),
                                stop=(c == cs - 1),
                            )
                        nc.scalar.copy(qkT[:, f, n * NQ:(n + 1) * NQ], ps[:, :])

            # ---- v projection, token-major per (image, token-tile)
            with tc.tile_pool(name="ps_v", bufs=4, space="PSUM") as ps_v:
                for b in range(BL):
                    for t in range(2):
                        m = 128 if t == 0 else N - 128
                        tok0 = b * N + t * 128
                        bt = b * 2 + t
                        for n in range(2):
                            ps = ps_v.tile([128, PRJ], F32)
                            for c in range(C):
                                nc.tensor.matmul(
                                    ps[0:m, :],
                                    lhsT=xt[:, c, tok0:tok0 + m],
                                    rhs=wv[:, c, n * PRJ:(n + 1) * PRJ],
                                    start=(c == 0),
                                    stop=(c == C - 1),
                                )
                            nc.scalar.copy(
                                vst[0:m, bt, n * 6:(n + 1) * 6, 0:HD],
                                ps[0:m, :].rearrange("p (h d) -> p h d", d=HD),
                            )
                        nc.gpsimd.memset(vst[:, bt, :, HD:HD + 1], 1.0)

            # ---- attention + output projection, per image
            with (
                tc.tile_pool(name="ps_s", bufs=3, space="PSUM") as ps_s,
                tc.tile_pool(name="ps_o", bufs=3, space="PSUM") as ps_o,
                tc.tile_pool(name="ps_p", bufs=2, space="PSUM") as ps_p,
            ):
                for b in range(BL):
                    q0 = b * N
                    for hp in range(H // 2):
                        pair = (2 * hp, 2 * hp + 1)
                        ss, es, os_ = {}, {}, {}
                        # scoresT = k @ q.T; the relative-position bias is
                        # folded in as exp(bias) multiplied after the exp.
                        # The second matmul runs M=128 using keys q0+128 ..
                        # q0+256 (spills into next image / zero tail — rows
                        # 69:128 are never consumed) so the accumulation group
                        # closes over all 128 partitions. Even/odd heads sit
                        # on complementary PE row groups, so adjacent emission
                        # lets their K=64 matmuls overlap in the array.
                        for h in pair:
                            ss[h] = ps_s.tile([128, N2], F32, tag="s", name=f"s_{b}_{h}")
                            po, fq, fk = (h % 2) * 64, h // 2, FT // 2 + h // 2
                            nc.tensor.matmul(
                                ss[h][0:128, 0:N],
                                lhsT=qkT[po:po + 64, fk, q0:q0 + 128],
                                rhs=qkT[po:po + 64, fq, q0:q0 + N],
                                start=True, stop=False,
                            )
                        for h in pair:
                            po, fq, fk = (h % 2) * 64, h // 2, FT // 2 + h // 2
                            nc.tensor.matmul(
                                ss[h][0:128, N:N2],
                                lhsT=qkT[po:po + 64, fk, q0 + 128:q0 + 256],
                                rhs=qkT[po:po + 64, fq, q0:q0 + N],
                                start=False, stop=True,
                            )
                        for h in pair:
                            es[h] = sb_e.tile([128, N2], BF16, tag="e", name=f"e_{b}_{h}")
                            nc.scalar.activation(
                                es[h][:, :], ss[h][:, :],
                                mybir.ActivationFunctionType.Exp,
                            )
                        for h in pair:
                            if h % 2 == 0:
                                nc.vector.tensor_mul(
                                    es[h][:, :], es[h][:, :], eb[:, h, :]
                                )
                            else:
                                nc.gpsimd.tensor_mul(
                                    es[h][:, :], es[h][:, :], eb[:, h, :]
                                )
                        # out.T (64 rows) + softmax denominators (row 64)
                        for h in pair:
                            os_[h] = ps_o.tile([128, N], F32, tag="o", name=f"o_{b}_{h}")
                            nc.tensor.matmul(
                                os_[h][0:HD + 1, :], lhsT=vst[:, b * 2, h, :],
                                rhs=es[h][0:128, 0:N], start=True, stop=False,
                            )
                            nc.tensor.matmul(
                                os_[h][0:HD + 1, :],
                                lhsT=vst[0:69, b * 2 + 1, h, :],
                                rhs=es[h][0:69, N:N2], start=False, stop=True,
                            )
                        for h in pair:
                            po, fq = (h % 2) * 64, h // 2
                            o = os_[h]
                            # custom-DVE reciprocal misreads PSUM; stage the
                            # denom row in SBUF first (cross-window copy is ok)
                            rc = sb_r.tile([1, N], F32, tag="rc")
                            nc.vector.tensor_copy(rc[0:1, :], o[64:65, :])
                            rr = sb_r.tile([1, N], F32, tag="rr")
                            nc.vector.reciprocal_approx_fast(rr[0:1, :], rc[0:1, :])
                            rb = sb_rb.tile([64, N], F32)
                            nc.gpsimd.partition_broadcast(rb[0:64, :], rr[0:1, :])
                            nc.vector.scalar_tensor_tensor(
                                out=aoT[po:po + 64, fq, q0:q0 + N],
                                in0=o[0:64, :], scalar=1.0, in1=rb[0:64, :],
                                op0=MUL, op1=MUL,
                            )
                            if debug_taps and b == 0 and h == 0:
                                nc.sync.dma_start(dbg_e[0:69, :], es[h][0:69, :])
                                nc.sync.dma_start(dbg_r[0:64, :], rb[0:64, :])

                    # output projection: 128-token tiles (batch-agnostic — aoT
                    # is flat over tokens); emit each tile once every image it
                    # spans is done
                    for j in range((TOK + 127) // 128):
                        tok0 = j * 128
                        m = min(128, TOK - tok0)
                        if (tok0 + m - 1) // N != b:
                            continue
                        ob = sb_out.tile([128, DIM], F32)
                        for n in range(2):
                            ps = ps_p.tile([128, PRJ], F32)
                            for c in range(C):
                                nc.tensor.matmul(
                                    ps[0:m, :],
                                    lhsT=aoT[:, c, tok0:tok0 + m],
                                    rhs=wp[:, c, n * PRJ:(n + 1) * PRJ],
                                    start=(c == 0),
                                    stop=(c == C - 1),
                                )
                            nc.scalar.copy(ob[0:m, n * PRJ:(n + 1) * PRJ], ps[0:m, :])
                        nc.sync.dma_start(out_d[tok0:tok0 + m, :], ob[0:m, :])

                if debug_taps:
                    nc.sync.dma_start(dbg_qkT[:], qkT[:])
                    nc.sync.dma_start(dbg_vst[:], vst[:])
                    nc.sync.dma_start(dbg_aoT[:], aoT[:])

    nc.finalize()
    return nc


def prep_shared(w_qkv, q_bias, v_bias, rel_table, w_proj, b_proj, rel_index):
    """Host-side weight/bias layouts shared by all cores (bf16)."""
    bf = ml_dtypes.bfloat16
    scale = HD ** -0.5

    wqk = np.zeros((CP, 2 * DIM), np.float32)
    wqk[0:DIM, 0:DIM] = w_qkv[0:DIM].T * scale
    wqk[DIM, 0:DIM] = q_bias * scale
    wqk[0:DIM, DIM:2 * DIM] = w_qkv[DIM:2 * DIM].T

    wv = np.zeros((CP, DIM), np.float32)
    wv[0:DIM] = w_qkv[2 * DIM:3 * DIM].T
    wv[DIM] = v_bias

    wp = np.zeros((CP, DIM), np.float32)
    wp[0:DIM] = w_proj.T
    wp[DIM] = b_proj

    # bias[q, k, h] -> key-major pair layout bp[key%128, h, (key<128 ? q : N+q)]
    bmat = rel_table[rel_index]          # [197(q), 197(k), 12]
    bp = np.zeros((128, H, N2), np.float32)
    bp[:, :, 0:N] = bmat[:, 0:128, :].transpose(1, 2, 0)
    bp[0:69, :, N:N2] = bmat[:, 128:N, :].transpose(1, 2, 0)

    return {
        "wqk": wqk.astype(bf),
        "wv": wv.astype(bf),
        "wp": wp.astype(bf),
        "bpair": bp.astype(bf),
        "ebpair": np.exp(bp).astype(bf),
    }


def prep_core_x(x, core):
    bf = ml_dtypes.bfloat16
    xs = x[core * BL:(core + 1) * BL].reshape(TOK, DIM)
    xt = np.zeros((CP, TOK), np.float32)
    xt[0:DIM] = xs.T
    xt[DIM] = 1.0
    return xt.astype(bf)


_built = None


def kernel(**inputs) -> np.ndarray:
    global _built
    from concourse.bass_utils import run_bass_kernel_spmd

    x = np.asarray(inputs["x"], np.float32)
    shared = prep_shared(
        np.asarray(inputs["w_qkv"], np.float32),
        np.asarray(inputs["q_bias"], np.float32),
        np.asarray(inputs["v_bias"], np.float32),
        np.asarray(inputs["rel_table"], np.float32),
        np.asarray(inputs["w_proj"], np.float32),
        np.asarray(inputs["b_proj"], np.float32),
        np.asarray(inputs["rel_index"], np.int32),
    )
    in_maps = [dict(shared, xt=prep_core_x(x, i)) for i in range(NCORES)]

    if _built is None:
        _built = build_module()
    res = run_bass_kernel_spmd(_built, in_maps, core_ids=list(range(NCORES)))
    out = np.concatenate(
        [np.asarray(res.results[i]["out"]).reshape(BL, N, DIM) for i in range(NCORES)],
        axis=0,
    )
    return out.astype(np.float32)


if __name__ == "__main__":
    nc = build_module()
    print("build OK:", len(nc.m.functions[0].blocks[0].instructions), "instructions?")


# revision 50
# speedup vs baseline: 1.1166x; 1.1166x over previous
"""Self-contained Trainium2 kernel for nn_Attention_22814866276679.

Multi-head attention (ViT-style, 197 tokens, 12 heads, dim 768) with a
relative-position bias table, batch 64. Data-parallel over batch across the
8 NeuronCores (8 images per core, no collectives).

Math notes (host prep moves all layout work off the device):
  - qkv = x @ w_qkv.T + concat(q_bias, 0, v_bias); q *= 1/8. The 1/8 scale
    and the biases are folded into pre-transposed weight matrices with an
    extra contraction row (x gets a ones row).
  - scores are computed TRANSPOSED ([keys, queries]) so the softmax reduce
    (over keys) lands on the matmul contraction axis; no PE transposes.
  - |scores + bias| <= ~3 for these inputs, so exp() is computed without the
    max-subtraction (mathematically identical softmax).
  - V carries an appended ones column: the attention@V matmul then emits the
    softmax denominators as a 65th output row for free.
"""

import os
import sys

for _p in ("/opt/trn_rl_repo", "/root/.axon_site/_ro/trn_rl_repo"):
    if os.path.isdir(_p) and _p not in sys.path:
        sys.path.insert(0, _p)

import ml_dtypes
import numpy as np

import concourse.bass as bass
import concourse.mybir as mybir
import concourse.tile as tile
from concourse import bacc, library_config
from concourse.masks import make_identity

BF16 = mybir.dt.bfloat16
F32 = mybir.dt.float32

B, N, DIM, H, HD = 64, 197, 768, 12, 64
NCORES = 8
BL = B // NCORES          # 8 images per core
TOK = BL * N              # 1576 tokens per core
C = 7                     # contraction chunks of 128 (768 dims + bias row, padded)
CP = C * 128              # 896
NQ = 394                  # qk-projection free chunk (4 * 394 = 1576)
PRJ = 384                 # v / output-projection free chunk (2 * 384 = 768)
N2 = 2 * N                # paired scores free size (keys 0:128 | keys 128:197)
FT = 2 * DIM // 128       # 12 q/k feature tiles (0-5: q, 6-11: k)

MUL = mybir.AluOpType.mult


def build_module(debug_taps: bool = False) -> bass.Bass:
    nc = bacc.Bacc()
    xt_d = nc.declare_dram_parameter("xt", [CP, TOK], BF16, isOutput=False)
    wqk_d = nc.declare_dram_parameter("wqk", [CP, 2 * DIM], BF16, isOutput=False)
    wv_d = nc.declare_dram_parameter("wv", [CP, DIM], BF16, isOutput=False)
    wp_d = nc.declare_dram_parameter("wp", [CP, DIM], BF16, isOutput=False)
    bp_d = nc.declare_dram_parameter("bpair", [128, H, N2], BF16, isOutput=False)
    out_d = nc.declare_dram_parameter("out", [TOK, DIM], F32, isOutput=True)
    if debug_taps:
        dbg_qkT = nc.declare_dram_parameter("dbg_qkT", [128, FT, TOK], BF16, isOutput=True)
        dbg_vst = nc.declare_dram_parameter("dbg_vst", [128, 2 * BL, H, HD + 1], BF16, isOutput=True)
        dbg_aoT = nc.declare_dram_parameter("dbg_aoT", [128, C, TOK], BF16, isOutput=True)
        dbg_e = nc.declare_dram_parameter("dbg_e", [128, N2], BF16, isOutput=True)
        dbg_r = nc.declare_dram_parameter("dbg_r", [128, N], F32, isOutput=True)

    with tile.TileContext(nc) as tc:
        with (
            tc.tile_pool(name="persist", bufs=1) as persist,
            tc.tile_pool(name="sb_e", bufs=6) as sb_e,
            tc.tile_pool(name="sb_r", bufs=8) as sb_r,
            tc.tile_pool(name="sb_rb", bufs=8) as sb_rb,
            tc.tile_pool(name="sb_out", bufs=4) as sb_out,
        ):
            xt = persist.tile([128, C, TOK], BF16)
            wqk = persist.tile([128, C, 2 * DIM], BF16)
            wv = persist.tile([128, C, DIM], BF16)
            wp = persist.tile([128, C, DIM], BF16)
            bp = persist.tile([128, H, N2], BF16)
            # f 0-5: qT, 6-11: kT; +64 zero tail columns let the second
            # scores matmul always run M=128 (keys q0+128 .. q0+256)
            qkT = persist.tile([128, FT, TOK + 64], BF16)
            vst = persist.tile([128, 2 * BL, H, HD + 1], BF16)
            aoT = persist.tile([128, C, TOK], BF16)
            ident = persist.tile([128, 128], BF16)

            make_identity(nc, ident[:, :])
            # partition_broadcast + gpsimd tensor_tensor live in 'proxy'
            nc.gpsimd.load_library(library_config.proxy)
            if debug_taps:
                nc.gpsimd.memset(vst[:], 0.0)
            # ones row (contraction row 768) for the output projection bias
            nc.gpsimd.memset(aoT[:, 6, :], 0.0)
            nc.gpsimd.memset(aoT[0:1, 6, :], 1.0)
            nc.gpsimd.memset(qkT[:, :, TOK:TOK + 64], 0.0)

            # per-chunk DMAs, interleaved so the first qk matmuls (which need
            # xt[c] + wqk[c]) can start as soon as their chunk lands
            for c in range(C):
                nc.sync.dma_start(xt[:, c, :], xt_d[c * 128:(c + 1) * 128, :])
                nc.sync.dma_start(wqk[:, c, :], wqk_d[c * 128:(c + 1) * 128, :])
            for c in range(C):
                nc.sync.dma_start(wv[:, c, :], wv_d[c * 128:(c + 1) * 128, :])
            nc.sync.dma_start(bp[:], bp_d[:])
            for c in range(C):
                nc.sync.dma_start(wp[:, c, :], wp_d[c * 128:(c + 1) * 128, :])

            # ---- q/k projections, feature-major: qkT[f] = w[f-block] @ x.T
            with tc.tile_pool(name="ps_qk", bufs=4, space="PSUM") as ps_qk:
                for f in range(FT):
                    # k features (f >= 6) have no bias: row 768 of wqk is zero
                    # there, so the c == 6 (bias/padding) chunk is skippable
                    cs = C if f < FT // 2 else C - 1
                    for n in range(TOK // NQ):
                        ps = ps_qk.tile([128, NQ], F32)
                        for c in range(cs):
                            nc.tensor.matmul(
                                ps[:, :],
                                lhsT=wqk[:, c, f * 128:(f + 1) * 128],
                                rhs=xt[:, c, n * NQ:(n + 1) * NQ],
                                start=(c == 0),
                                stop=(c == cs - 1),
                            )
                        nc.scalar.copy(qkT[:, f, n * NQ:(n + 1) * NQ], ps[:, :])

            # ---- v projection, token-major per (image, token-tile)
            with tc.tile_pool(name="ps_v", bufs=4, space="PSUM") as ps_v:
                for b in range(BL):
                    for t in range(2):
                        m = 128 if t == 0 else N - 128
                        tok0 = b * N + t * 128
                        bt = b * 2 + t
                        for n in range(2):
                            ps = ps_v.tile([128, PRJ], F32)
                            for c in range(C):
                                nc.tensor.matmul(
                                    ps[0:m, :],
                                    lhsT=xt[:, c, tok0:tok0 + m],
                                    rhs=wv[:, c, n * PRJ:(n + 1) * PRJ],
                                    start=(c == 0),
                                    stop=(c == C - 1),
                                )
                            nc.scalar.copy(
                                vst[0:m, bt, n * 6:(n + 1) * 6, 0:HD],
                                ps[0:m, :].rearrange("p (h d) -> p h d", d=HD),
                            )
                        nc.gpsimd.memset(vst[:, bt, :, HD:HD + 1], 1.0)

            # ---- attention + output projection, per image
            with (
                tc.tile_pool(name="ps_s", bufs=3, space="PSUM") as ps_s,
                tc.tile_pool(name="ps_o", bufs=3, space="PSUM") as ps_o,
                tc.tile_pool(name="ps_p", bufs=2, space="PSUM") as ps_p,
            ):
                for b in range(BL):
                    q0 = b * N
                    for hp in range(H // 2):
                        pair = (2 * hp, 2 * hp + 1)
                        ss, es, os_ = {}, {}, {}
                        # scoresT = biasT + k @ q.T in one PSUM bank per head.
                        # Bias matmul first (start=True, full tile); the second
                        # scores matmul runs M=128 using keys q0+128 .. q0+256
                        # (spills into next image / zero tail — rows 69:128 of
                        # that half are never consumed) so every matmul covers
                        # all 128 partitions and the group closes cleanly.
                        # Even/odd heads sit on complementary PE row groups,
                        # so adjacent emission lets their K=64 matmuls overlap.
                        for h in pair:
                            ss[h] = ps_s.tile([128, N2], F32, tag="s", name=f"s_{b}_{h}")
                            nc.tensor.matmul(
                                ss[h][:, :], lhsT=ident[:, :], rhs=bp[:, h, :],
                                start=True, stop=False,
                            )
                        for h in pair:
                            po, fq, fk = (h % 2) * 64, h // 2, FT // 2 + h // 2
                            nc.tensor.matmul(
                                ss[h][0:128, N:N2],
                                lhsT=qkT[po:po + 64, fk, q0 + 128:q0 + 256],
                                rhs=qkT[po:po + 64, fq, q0:q0 + N],
                                start=False, stop=False,
                            )
                        for h in pair:
                            po, fq, fk = (h % 2) * 64, h // 2, FT // 2 + h // 2
                            nc.tensor.matmul(
                                ss[h][0:128, 0:N],
                                lhsT=qkT[po:po + 64, fk, q0:q0 + 128],
                                rhs=qkT[po:po + 64, fq, q0:q0 + N],
                                start=False, stop=True,
                            )
                        for h in pair:
                            es[h] = sb_e.tile([128, N2], BF16, tag="e", name=f"e_{b}_{h}")
                            nc.scalar.activation(
                                es[h][:, :], ss[h][:, :],
                                mybir.ActivationFunctionType.Exp,
                            )
                        # out.T (64 rows) + softmax denominators (row 64)
                        for h in pair:
                            os_[h] = ps_o.tile([128, N], F32, tag="o", name=f"o_{b}_{h}")
                            nc.tensor.matmul(
                                os_[h][0:HD + 1, :], lhsT=vst[:, b * 2, h, :],
                                rhs=es[h][0:128, 0:N], start=True, stop=False,
                            )
                            nc.tensor.matmul(
                                os_[h][0:HD + 1, :],
                                lhsT=vst[0:69, b * 2 + 1, h, :],
                                rhs=es[h][0:69, N:N2], start=False, stop=True,
                            )
                        for h in pair:
                            po, fq = (h % 2) * 64, h // 2
                            o = os_[h]
                            # custom-DVE reciprocal misreads PSUM; stage the
                            # denom row in SBUF first (cross-window copy is ok)
                            rc = sb_r.tile([1, N], F32, tag="rc")
                            nc.vector.tensor_copy(rc[0:1, :], o[64:65, :])
                            rr = sb_r.tile([1, N], F32, tag="rr")
                            nc.vector.reciprocal_approx_fast(rr[0:1, :], rc[0:1, :])
                            rb = sb_rb.tile([64, N], F32)
                            nc.gpsimd.partition_broadcast(rb[0:64, :], rr[0:1, :])
                            nc.vector.scalar_tensor_tensor(
                                out=aoT[po:po + 64, fq, q0:q0 + N],
                                in0=o[0:64, :], scalar=1.0, in1=rb[0:64, :],
                                op0=MUL, op1=MUL,
                            )
                            if debug_taps and b == 0 and h == 0:
                                nc.sync.dma_start(dbg_e[0:69, :], es[h][0:69, :])
                                nc.sync.dma_start(dbg_r[0:64, :], rb[0:64, :])

                    # output projection: 128-token tiles (batch-agnostic — aoT
                    # is flat over tokens); emit each tile once every image it
                    # spans is done
                    for j in range((TOK + 127) // 128):
                        tok0 = j * 128
                        m = min(128, TOK - tok0)
                        if (tok0 + m - 1) // N != b:
                            continue
                        ob = sb_out.tile([128, DIM], F32)
                        for n in range(2):
                            ps = ps_p.tile([128, PRJ], F32)
                            for c in range(C):
                                nc.tensor.matmul(
                                    ps[0:m, :],
                                    lhsT=aoT[:, c, tok0:tok0 + m],
                                    rhs=wp[:, c, n * PRJ:(n + 1) * PRJ],
                                    start=(c == 0),
                                    stop=(c == C - 1),
                                )
                            nc.scalar.copy(ob[0:m, n * PRJ:(n + 1) * PRJ], ps[0:m, :])
                        nc.sync.dma_start(out_d[tok0:tok0 + m, :], ob[0:m, :])

                if debug_taps:
                    nc.sync.dma_start(dbg_qkT[:], qkT[:])
                    nc.sync.dma_start(dbg_vst[:], vst[:])
                    nc.sync.dma_start(dbg_aoT[:], aoT[:])

    nc.finalize()
    return nc


def prep_shared(w_qkv, q_bias, v_bias, rel_table, w_proj, b_proj, rel_index):
    """Host-side weight/bias layouts shared by all cores (bf16)."""
    bf = ml_dtypes.bfloat16
    scale = HD ** -0.5

    wqk = np.zeros((CP, 2 * DIM), np.float32)
    wqk[0:DIM, 0:DIM] = w_qkv[0:DIM].T * scale
    wqk[DIM, 0:DIM] = q_bias * scale
    wqk[0:DIM, DIM:2 * DIM] = w_qkv[DIM:2 * DIM].T

    wv = np.zeros((CP, DIM), np.float32)
    wv[0:DIM] = w_qkv[2 * DIM:3 * DIM].T
    wv[DIM] = v_bias

    wp = np.zeros((CP, DIM), np.float32)
    wp[0:DIM] = w_proj.T
    wp[DIM] = b_proj

    # bias[q, k, h] -> key-major pair layout bp[key%128, h, (key<128 ? q : N+q)]
    bmat = rel_table[rel_index]          # [197(q), 197(k), 12]
    bp = np.zeros((128, H, N2), np.float32)
    bp[:, :, 0:N] = bmat[:, 0:128, :].transpose(1, 2, 0)
    bp[0:69, :, N:N2] = bmat[:, 128:N, :].transpose(1, 2, 0)

    return {
        "wqk": wqk.astype(bf),
        "wv": wv.astype(bf),
        "wp": wp.astype(bf),
        "bpair": bp.astype(bf),
    }


def prep_core_x(x, core):
    bf = ml_dtypes.bfloat16
    xs = x[core * BL:(core + 1) * BL].reshape(TOK, DIM)
    xt = np.zeros((CP, TOK), np.float32)
    xt[0:DIM] = xs.T
    xt[DIM] = 1.0
    return xt.astype(bf)


_built = None


def kernel(**inputs) -> np.ndarray:
    global _built
    from concourse.bass_utils import run_bass_kernel_spmd

    x = np.asarray(inputs["x"], np.float32)
    shared = prep_shared(
        np.asarray(inputs["w_qkv"], np.float32),
        np.asarray(inputs["q_bias"], np.float32),
        np.asarray(inputs["v_bias"], np.float32),
        np.asarray(inputs["rel_table"], np.float32),
        np.asarray(inputs["w_proj"], np.float32),
        np.asarray(inputs["b_proj"], np.float32),
        np.asarray(inputs["rel_index"], np.int32),
    )
    in_maps = [dict(shared, xt=prep_core_x(x, i)) for i in range(NCORES)]

    if _built is None:
        _built = build_module()
    res = run_bass_kernel_spmd(_built, in_maps, core_ids=list(range(NCORES)))
    out = np.concatenate(
        [np.asarray(res.results[i]["out"]).reshape(BL, N, DIM) for i in range(NCORES)],
        axis=0,
    )
    return out.astype(np.float32)


if __name__ == "__main__":
    nc = build_module()
    print("build OK:", len(nc.m.functions[0].blocks[0].instructions), "instructions?")
